# revision 2
# baseline (speedup 1.0000x reference)
"""3-layer GCN (PyG gcn_norm semantics) on 8 Trainium2 NeuronCores — v2.

Design
------
Nodes dealt round-robin by in-degree across 8 cores; each core owns NSLOT
dest slots (50 groups of 128). With v := D x (D = deg^-1/2 pre-scale):

  per layer l:  g = v W_l          (feature-major matmul on PE, own block)
                AllGather g (bf16) -> node-major table g_buf[TOT, H]
                acc[d] = g[d] + sum_{in-edges} g[src]
                         (self term from local gnode tile; edge terms via
                          dma_gather rounds + PE identity-matmul into PSUM)
                v' = relu(dinv*(dinv*acc + b))

Layer 1 is sharded exactly like the rest (x pre-scaled/transposed on host).
int16 gather indices cover 32766 rows < TOT=52224, so two overlapping
windows [0, 32766) and [TOT-32766, TOT); each dest's in-edges are split
across the windows by a host-side waterfill, padded per (group, window) to
the group max with pointers to all-zero pad rows.
"""
import sys

sys.path.insert(0, "/opt/trn_rl_repo")

import numpy as np
import ml_dtypes

import concourse.bass as bass
import concourse.mybir as mybir
import concourse.tile as tile
import concourse.bacc as bacc
from concourse.bass_utils import run_bass_kernel_spmd

BF16 = ml_dtypes.bfloat16
NCORES = 8

N = 50000
NFEAT = 512
NHID = 128
NCLASS = 10

NG = 50                              # dest groups per core
NSLOT = NG * 128                     # dest slots per core (6400 >= 6250)
BR = NSLOT + 128                     # table block rows per core (zero tail)
TOT = BR * NCORES                    # 52224
TBL = 32766                          # int16-addressable window rows
TBASE = [0, TOT - TBL]               # [0, 19458]
NT = 2
N_SB = 10                            # sub-batches; each takes every 10th group


def _cumcount(sorted_keys):
    n = sorted_keys.shape[0]
    first = np.ones(n, dtype=bool)
    first[1:] = sorted_keys[1:] != sorted_keys[:-1]
    idx_of_first = np.maximum.accumulate(np.where(first, np.arange(n), 0))
    return np.arange(n) - idx_of_first


def preprocess(edge_index):
    src = np.asarray(edge_index[0], dtype=np.int64)
    dst = np.asarray(edge_index[1], dtype=np.int64)
    E = src.shape[0]

    deg = np.bincount(dst, minlength=N).astype(np.int64) + 1   # + self loop
    dinv = (1.0 / np.sqrt(deg.astype(np.float32))).astype(np.float32)

    order = np.argsort(-deg, kind="stable")
    core_of = np.empty(N, np.int64)
    slot_of = np.empty(N, np.int64)
    r = np.arange(N)
    core_of[order] = r % NCORES
    slot_of[order] = r // NCORES

    e_core = core_of[dst]
    e_slot = slot_of[dst]
    e_row = core_of[src] * BR + slot_of[src]       # source row in g_buf

    # window regions: 0 = A-only, 1 = flex, 2 = B-only
    lo1 = TBASE[1]
    region = np.full(E, 1, np.int8)
    region[e_row < lo1] = 0
    region[e_row >= TBL] = 2

    gdest = e_core * NSLOT + e_slot
    NDEST = NCORES * NSLOT
    cnt = np.zeros((3, NDEST), np.int64)
    for rr in range(3):
        cnt[rr] = np.bincount(gdest[region == rr], minlength=NDEST)
    k = cnt.sum(0)

    # waterfill: flex edges top window A up to ceil(k/2)
    takeA = np.clip((k + 1) // 2 - cnt[0], 0, cnt[1])
    nA = cnt[0] + takeA
    nB = k - nA

    o = np.lexsort((e_row, region, gdest))
    rnk = _cumcount(gdest[o] * 4 + region[o])
    tb = np.empty(E, np.int8)
    og, orr = gdest[o], region[o]
    tb[orr == 0] = 0
    tb[orr == 2] = 1
    m = orr == 1
    tb[m] = np.where(rnk[m] < takeA[og[m]], 0, 1)
    table = np.empty(E, np.int8)
    table[o] = tb

    # round index = rank within (dest, window), edges sorted by source row
    o2 = np.lexsort((e_row, table, gdest))
    rnk2 = _cumcount(gdest[o2] * 2 + table[o2])
    rounds = np.empty(E, np.int64)
    rounds[o2] = rnk2

    ntab = np.stack([nA, nB]).reshape(2, NCORES, NG, 128)
    Rgt = ntab.max(axis=(1, 3))                    # [2, NG]

    # stride composition balances per-sub-batch round totals (group 0 has the
    # highest-degree dests, group NG-1 the lowest)
    sbs = [list(range(i, NG, N_SB)) for i in range(N_SB)]

    # zero pad rows local to each window (block tails are always zero)
    zero_local = []
    for t in range(NT):
        zr = None
        for b in range(NCORES):
            cand = b * BR + NSLOT
            if cand >= TBASE[t] and cand + 127 < TBASE[t] + TBL:
                zr = cand - TBASE[t]
                break
        assert zr is not None
        zero_local.append(zr)

    Rmax = max(int(Rgt.max()), 1)
    dense = np.zeros((NCORES, NT, NSLOT, Rmax), np.int64)
    for t in range(NT):
        dense[:, t] = zero_local[t]
    for t in range(NT):
        m = table == t
        local = e_row[m] - TBASE[t]
        assert local.min() >= 0 and local.max() < TBL
        dense[e_core[m], t, e_slot[m], rounds[m]] = local

    call_offsets = []
    total_words = 0
    for sb in sbs:
        for t in range(NT):
            ni = int(sum(Rgt[t, g] for g in sb)) * 128
            call_offsets.append((ni, total_words))
            total_words += ni // 16
    TOTW = total_words

    idx_arr = np.zeros((NCORES, 128, TOTW), np.int16)
    ci = 0
    for sb in sbs:
        for t in range(NT):
            ni, off = call_offsets[ci]; ci += 1
            if ni == 0:
                continue
            vals = np.concatenate(
                [dense[:, t, g * 128:(g + 1) * 128, :Rgt[t, g]].transpose(0, 2, 1)
                 .reshape(NCORES, -1) for g in sb if Rgt[t, g] > 0], axis=1)
            assert vals.shape[1] == ni
            w = vals.reshape(NCORES, ni // 16, 16).transpose(0, 2, 1)
            for rep in range(8):
                idx_arr[:, rep * 16:(rep + 1) * 16, off:off + ni // 16] = w

    dinv_slot = np.zeros((NCORES, NSLOT), np.float32)
    dinv_slot[core_of, slot_of] = dinv

    node_of = np.full((NCORES, NSLOT), -1, np.int64)
    node_of[core_of, slot_of] = np.arange(N)

    return dict(
        dinv=dinv, core_of=core_of, slot_of=slot_of, node_of=node_of,
        Rgt=Rgt, sbs=sbs, call_offsets=call_offsets, TOTW=TOTW,
        idx_arr=idx_arr, dinv_slot=dinv_slot,
    )


def build_kernel(meta, n_layers=3, do_collective=True, do_gather=True):
    Rgt, sbs, call_offsets, TOTW = meta["Rgt"], meta["sbs"], meta["call_offsets"], meta["TOTW"]
    f32, bf16, i16 = mybir.dt.float32, mybir.dt.bfloat16, mybir.dt.int16
    AF = mybir.ActivationFunctionType
    KCH = NFEAT // 128

    nc = bacc.Bacc("TRN2", target_bir_lowering=False, debug=False,
                   num_devices=NCORES)

    xt_in = nc.declare_dram_parameter("xt", [128, KCH, NSLOT], bf16, isOutput=False)
    w1_in = nc.declare_dram_parameter("w1", [128, KCH, NHID], bf16, isOutput=False)
    w2_in = nc.declare_dram_parameter("w2", [128, NHID], bf16, isOutput=False)
    w3_in = nc.declare_dram_parameter("w3", [128, NHID], bf16, isOutput=False)
    wl_in = nc.declare_dram_parameter("wl", [128, NCLASS], f32, isOutput=False)
    dinv_in = nc.declare_dram_parameter("dinv", [128, NG], f32, isOutput=False)
    bias_in = nc.declare_dram_parameter("bias", [128, 2, NHID], f32, isOutput=False)
    bprime_in = nc.declare_dram_parameter("bprime", [128, NCLASS], f32, isOutput=False)
    idb_in = nc.declare_dram_parameter("idb", [128, 128], bf16, isOutput=False)
    idf_in = nc.declare_dram_parameter("idf", [128, 128], f32, isOutput=False)
    idx_in = nc.declare_dram_parameter("gidx", [128, TOTW], i16, isOutput=False)
    out_ext = nc.declare_dram_parameter("out", [NSLOT, NCLASS], f32, isOutput=True)

    NCHUNK = [(i * 512, min(512, NSLOT - i * 512)) for i in range((NSLOT + 511) // 512)]

    with tile.TileContext(nc) as tc:
        with (
            tc.tile_pool(name="dram", bufs=1, space="DRAM") as dramp,
            tc.tile_pool(name="const", bufs=1) as constp,
            tc.tile_pool(name="vbig", bufs=1) as vbigp,
            tc.tile_pool(name="xtp", bufs=2) as xtp,
            tc.tile_pool(name="mm", bufs=4) as mmp,
            tc.tile_pool(name="epi", bufs=4) as epip,
            tc.tile_pool(name="gt", bufs=2) as gtp,
            tc.tile_pool(name="psA", bufs=2, space="PSUM") as psA,
            tc.tile_pool(name="psT", bufs=3, space="PSUM") as psT,
            tc.tile_pool(name="psC", bufs=3, space="PSUM") as psC,
        ):
            ag_ins = [dramp.tile([BR, NHID], bf16, name=f"ag_in{l}") for l in range(3)]
            g_bufs = [dramp.tile([TOT, NHID], bf16, addr_space="Shared",
                                 name=f"g_buf{l}") for l in range(3)]
            # ---- constants ----
            w1_sb = constp.tile([128, KCH, NHID], bf16, name="w1_sb")
            nc.sync.dma_start(w1_sb[:], w1_in[:])
            w2_sb = constp.tile([128, NHID], bf16, name="w2_sb")
            nc.sync.dma_start(w2_sb[:], w2_in[:])
            w3_sb = constp.tile([128, NHID], bf16, name="w3_sb")
            nc.sync.dma_start(w3_sb[:], w3_in[:])
            wl_sb = constp.tile([128, NCLASS], f32, name="wl_sb")
            nc.sync.dma_start(wl_sb[:], wl_in[:])
            dinv_sb = constp.tile([128, NG], f32, name="dinv_sb")
            nc.sync.dma_start(dinv_sb[:], dinv_in[:])
            bias_sb = constp.tile([128, 2, NHID], f32, name="bias_sb")
            nc.sync.dma_start(bias_sb[:], bias_in[:])
            bprime_sb = constp.tile([128, NCLASS], f32, name="bprime_sb")
            nc.sync.dma_start(bprime_sb[:], bprime_in[:])
            idb_sb = constp.tile([128, 128], bf16, name="idb_sb")
            nc.sync.dma_start(idb_sb[:], idb_in[:])
            idf_sb = constp.tile([128, 128], f32, name="idf_sb")
            nc.sync.dma_start(idf_sb[:], idf_in[:])
            idx_sb = constp.tile([128, TOTW], i16, name="idx_sb")
            nc.sync.dma_start(idx_sb[:], idx_in[:])

            # zero the pad-row tails of ag_in (rows NSLOT..BR) once
            zpad = constp.tile([128, NHID], bf16, name="zpad")
            nc.vector.memset(zpad[:], 0.0)
            for l in range(3):
                nc.sync.dma_start(
                    ag_ins[l][NSLOT:BR, :].rearrange("(a p) f -> p a f", p=128),
                    zpad.rearrange("p (a f) -> p a f", a=1))

            vT = None  # feature-major v^T of previous layer [128, NSLOT] bf16

            for layer in range(n_layers):
                ag_in = ag_ins[layer]
                g_buf = g_bufs[layer]

                # ============ phase A: g = v W (node-major gnode blocks) =====
                gnode = vbigp.tile([128, NSLOT], bf16, name="gnode", tag="gnode",
                                   bufs=2)
                for (c0, cn) in NCHUNK:
                    pg = psA.tile([128, 512], f32, name="pg", tag="pg")
                    if layer == 0:
                        xtf = xtp.tile([128, KCH, 512], bf16, name="xtf", tag="xtf")
                        nc.sync.dma_start(xtf[:, :, 0:cn], xt_in[:, :, c0:c0 + cn])
                        for kk in range(KCH):
                            nc.tensor.matmul(pg[:, 0:cn], w1_sb[:, kk, :],
                                             xtf[:, kk, 0:cn],
                                             start=(kk == 0), stop=(kk == KCH - 1))
                    else:
                        wsb = w2_sb if layer == 1 else w3_sb
                        nc.tensor.matmul(pg[:, 0:cn], wsb[:],
                                         vT[:, c0:c0 + cn], start=True, stop=True)
                    for j in range(cn // 128):
                        g = (c0 + j * 128) // 128
                        mm_sb = mmp.tile([128, 128], f32, name="mm_sb", tag="mm")
                        nc.scalar.activation(mm_sb[:], pg[:, j * 128:(j + 1) * 128],
                                             AF.Copy)
                        ptb = psT.tile([128, 128], f32, name="ptb", tag="pt")
                        nc.tensor.transpose(ptb[:], mm_sb[:], idf_sb[:])
                        nc.scalar.activation(gnode[:, g * 128:(g + 1) * 128],
                                             ptb[:], AF.Copy)
                nc.sync.dma_start(
                    ag_in[0:NSLOT, :].rearrange("(g p) f -> p g f", p=128),
                    gnode.rearrange("p (g f) -> p g f", f=NHID))

                # ============ phase B: AllGather ==============================
                if do_collective:
                    nc.gpsimd.collective_compute(
                        "AllGather", mybir.AluOpType.bypass,
                        replica_groups=[list(range(NCORES))],
                        ins=[ag_in[:]], outs=[g_buf[:]],
                    )
                else:
                    nc.sync.dma_start(
                        g_buf[0:NSLOT, :].rearrange("(g p) f -> p g f", p=128),
                        gnode.rearrange("p (g f) -> p g f", f=NHID))

                # ============ phase C: gather + accumulate + epilogue =========
                vTn = (vbigp.tile([128, NSLOT], bf16, name="vTn", tag="vT", bufs=2)
                       if layer < 2 else None)
                outbig = (vbigp.tile([128, NG * NCLASS], f32, name="outbig")
                          if layer == 2 else None)
                ci = 0
                for sb in sbs:
                    tiles = []
                    offs = []
                    for t in range(NT):
                        ni, off = call_offsets[ci]; ci += 1
                        R = max(ni // 128, 1)
                        gt = gtp.tile([128, R * NHID], bf16, name=f"gt{t}",
                                      tag=f"gt{t}")
                        if do_gather and ni > 0:
                            nc.gpsimd.dma_gather(
                                gt.rearrange("p (r f) -> p r f", f=NHID),
                                g_buf[TBASE[t]:TBASE[t] + TBL, :],
                                idx_sb[:, off:off + ni // 16],
                                ni, ni, NHID, single_packet=False,
                            )
                        elif ni > 0:
                            nc.vector.memset(gt[:], 0.0)
                        tiles.append(gt)
                        co = np.concatenate([[0], np.cumsum([Rgt[t, g] for g in sb])])
                        offs.append(co)
                    for gi, g in enumerate(sb):
                        acc = psC.tile([128, NHID], f32, name="acc", tag="acc")
                        rtot = int(Rgt[:, g].sum())
                        # self-loop contribution from the local gnode block
                        nc.tensor.matmul(acc[:], idb_sb[:],
                                         gnode[:, g * 128:(g + 1) * 128],
                                         start=True, stop=(rtot == 0))
                        done = 0
                        for t in range(NT):
                            gt = tiles[t].rearrange("p (r f) -> p r f", f=NHID)
                            for rr in range(int(Rgt[t, g])):
                                nc.tensor.matmul(
                                    acc[:], idb_sb[:],
                                    gt[:, int(offs[t][gi]) + rr, :],
                                    start=False, stop=(done == rtot - 1))
                                done += 1
                        dcol = dinv_sb[:, g:g + 1]
                        if layer < 2:
                            t1 = epip.tile([128, NHID], f32, name="t1", tag="t1")
                            nc.scalar.activation(t1[:], acc[:], AF.Copy, scale=dcol)
                            t2 = epip.tile([128, NHID], f32, name="t2", tag="t2")
                            nc.vector.tensor_add(t2[:], t1[:], bias_sb[:, layer, :])
                            vn = epip.tile([128, NHID], bf16, name="vn", tag="vn")
                            nc.scalar.activation(vn[:], t2[:], AF.Relu, scale=dcol)
                            pt2 = psT.tile([128, 128], bf16, name="pt2", tag="pt")
                            nc.tensor.transpose(pt2[:], vn[:], idb_sb[:])
                            nc.scalar.activation(vTn[:, g * 128:(g + 1) * 128],
                                                 pt2[:], AF.Copy)
                        else:
                            t1 = epip.tile([128, NHID], f32, name="t1", tag="t1")
                            nc.scalar.activation(t1[:], acc[:], AF.Copy, scale=dcol)
                            pt2 = psT.tile([128, 128], f32, name="pt2", tag="pt")
                            nc.tensor.transpose(pt2[:], t1[:], idf_sb[:])
                            t1T = epip.tile([128, NHID], f32, name="t1T", tag="t1T")
                            nc.scalar.activation(t1T[:], pt2[:], AF.Copy)
                            pl = psT.tile([128, NCLASS], f32, name="pl", tag="pt")
                            nc.tensor.matmul(pl[:], t1T[:], wl_sb[:],
                                             start=True, stop=True)
                            lg = epip.tile([128, NCLASS], f32, name="lg", tag="lg")
                            nc.vector.tensor_add(lg[:], pl[:], bprime_sb[:])
                            mx = epip.tile([128, 1], f32, name="mx", tag="mx")
                            nc.vector.tensor_reduce(mx[:], lg[:],
                                                    mybir.AxisListType.X,
                                                    mybir.AluOpType.max, negate=True)
                            ex = epip.tile([128, NCLASS], f32, name="ex", tag="ex")
                            nc.scalar.activation(ex[:], lg[:], AF.Exp, bias=mx[:])
                            sm = epip.tile([128, 1], f32, name="sm", tag="sm")
                            nc.vector.tensor_reduce(sm[:], ex[:],
                                                    mybir.AxisListType.X,
                                                    mybir.AluOpType.add)
                            ls = epip.tile([128, 1], f32, name="ls", tag="ls")
                            nc.scalar.activation(ls[:], sm[:], AF.Ln)
                            adj = epip.tile([128, 1], f32, name="adj", tag="adj")
                            nc.vector.tensor_sub(adj[:], mx[:], ls[:])
                            nc.vector.tensor_scalar_add(
                                outbig[:, g * NCLASS:(g + 1) * NCLASS], lg[:], adj[:])
                if layer < 2 and layer < n_layers - 1:
                    vT = vTn
                elif layer < 2:
                    # debug partial build: dump vTn slice so the kernel has output
                    dbg = vbigp.tile([128, NG * NCLASS], f32, name="dbgout")
                    for g in range(NG):
                        nc.scalar.activation(dbg[:, g * NCLASS:(g + 1) * NCLASS],
                                             vTn[:, g * 128:g * 128 + NCLASS],
                                             AF.Copy)
                    nc.sync.dma_start(
                        out_ext.rearrange("(g p) c -> p g c", p=128),
                        dbg.rearrange("p (g c) -> p g c", c=NCLASS))
                else:
                    nc.sync.dma_start(
                        out_ext.rearrange("(g p) c -> p g c", p=128),
                        outbig.rearrange("p (g c) -> p g c", c=NCLASS))

    nc.compile()
    return nc


def build_in_maps(meta, x, W1, b1, W2, b2, W3, b3, Wl, bl):
    dinv, node_of = meta["dinv"], meta["node_of"]
    b1 = np.asarray(b1, np.float32)
    bias = np.stack([np.tile(b1, (128, 1)),
                     np.tile(np.asarray(b2, np.float32), (128, 1))],
                    axis=1).astype(np.float32)
    bprime = np.tile(b3 @ Wl + bl, (128, 1)).astype(np.float32)
    idb = np.eye(128, dtype=np.float32).astype(BF16)
    idf = np.eye(128, dtype=np.float32)
    w1 = np.ascontiguousarray(
        W1.reshape(NFEAT // 128, 128, NHID).transpose(1, 0, 2)).astype(BF16)

    in_maps = []
    for c in range(NCORES):
        nodes = node_of[c]
        valid = nodes >= 0
        xs = np.zeros((NSLOT, NFEAT), np.float32)
        xs[valid] = x[nodes[valid]] * dinv[nodes[valid]][:, None]
        xt = np.ascontiguousarray(
            xs.T.reshape(NFEAT // 128, 128, NSLOT).transpose(1, 0, 2)).astype(BF16)
        dv = meta["dinv_slot"][c].reshape(NG, 128).T.astype(np.float32)
        in_maps.append({
            "xt": xt, "w1": w1,
            "w2": W2.astype(BF16), "w3": W3.astype(BF16), "wl": Wl,
            "dinv": np.ascontiguousarray(dv),
            "bias": bias, "bprime": bprime,
            "idb": idb, "idf": idf,
            "gidx": meta["idx_arr"][c],
        })
    return in_maps


_CACHE = {}


def kernel(x, edge_index, W1, b1, W2, b2, W3, b3, Wl, bl):
    x = np.asarray(x, np.float32)
    edge_index = np.asarray(edge_index)
    W1 = np.asarray(W1, np.float32); b1 = np.asarray(b1, np.float32)
    W2 = np.asarray(W2, np.float32); b2 = np.asarray(b2, np.float32)
    W3 = np.asarray(W3, np.float32); b3 = np.asarray(b3, np.float32)
    Wl = np.asarray(Wl, np.float32); bl = np.asarray(bl, np.float32)

    key = hash(edge_index.tobytes())
    if key not in _CACHE:
        meta = preprocess(edge_index)
        nc = build_kernel(meta)
        _CACHE[key] = (meta, nc)
    meta, nc = _CACHE[key]

    node_of = meta["node_of"]
    in_maps = build_in_maps(meta, x, W1, b1, W2, b2, W3, b3, Wl, bl)
    res = run_bass_kernel_spmd(nc, in_maps, list(range(NCORES)))

    out = np.empty((N, NCLASS), np.float32)
    for c in range(NCORES):
        o = res.results[c]["out"]
        nodes = node_of[c]
        valid = nodes >= 0
        out[nodes[valid]] = o[valid]
    return out
